# revision 5
# baseline (speedup 1.0000x reference)
"""Causal multi-head self-attention on 8 trn2 NeuronCores.

Problem (hardcoded): x [4, 2048, 1024] fp32, 16 heads, d_k = 64.
    q/k/v = x @ W{q,k,v}.T + b     (per-head split)
    attn  = softmax(causal(q k^T / 8)) @ v
    out   = concat_heads(attn) @ Wo.T + bo

Sharding: tensor-parallel over heads. Core j owns heads {2j, 2j+1} =
feature slice [128j, 128j+128). Each core:
  1. Q/K/V projections for its slice, all 8192 tokens (output kept
     transposed: [e_local, token] in SBUF).
  2. Flash-style causal attention for its 8 (batch, head) pairs,
     computed in scores^T layout ([key, query]) so the attention
     weights land contraction-major for the V matmul. Softmax
     denominator comes from a ones-column appended to V.
  3. Partial output projection  attn_out_slice @ Wo[:, slice].T
     -> [8192, 1024] partial, summed on the host over the 8 cores.

All matmuls run as float32r (TF32-like, full PE rate at N>=512).
"""

import contextlib
import ctypes
import sys
import types

import numpy as np

import concourse.bacc as bacc
import concourse.bass as bass
import concourse.mybir as mybir
import concourse.tile as tile
from concourse.bass_utils import run_bass_kernel_spmd
from concourse.masks import make_identity

F32 = mybir.dt.float32
F32R = mybir.dt.float32r
AF = mybir.ActivationFunctionType

B, S, D = 4, 2048, 1024
H, DK = 16, 64
T = B * S              # 8192 tokens
P = 128                # partitions
NCORES = 8
ESL = D // NCORES      # 128 = per-core feature slice (2 heads)
KD = D // P            # 8 k-tiles for the projections
PCH = 512              # projection token-chunk
NPCH = T // PCH        # 16
QC = 512               # query chunk in attention
NQC = S // QC          # 4 per batch
NST = T // P           # 64 token subtiles (for V natural layout)


def _build():
    nc = bacc.Bacc("TRN2", target_bir_lowering=False, debug=False)

    xT_d = nc.dram_tensor("xT", (D, T), F32, kind="ExternalInput")
    wqT_d = nc.dram_tensor("wqT", (D, ESL), F32, kind="ExternalInput")
    wkT_d = nc.dram_tensor("wkT", (D, ESL), F32, kind="ExternalInput")
    wvT_d = nc.dram_tensor("wvT", (D, ESL), F32, kind="ExternalInput")
    bq_d = nc.dram_tensor("bq", (ESL, 1), F32, kind="ExternalInput")
    bk_d = nc.dram_tensor("bk", (ESL, 1), F32, kind="ExternalInput")
    bv_d = nc.dram_tensor("bv", (ESL, 1), F32, kind="ExternalInput")
    woT_d = nc.dram_tensor("woT", (ESL, D), F32, kind="ExternalInput")
    ones_d = nc.dram_tensor("onesb", (P, NST), F32, kind="ExternalInput")
    out_d = nc.dram_tensor("outp", (T, D), F32, kind="ExternalOutput")

    with tile.TileContext(nc) as tc:
        with (
            tc.tile_pool(name="const", bufs=1) as const,
            tc.tile_pool(name="xs", bufs=2) as xs,
            tc.tile_pool(name="big", bufs=1) as big,
            tc.tile_pool(name="vt", bufs=2) as vtp,
            tc.tile_pool(name="attn", bufs=3) as attnp,
            tc.tile_pool(name="otp", bufs=2) as otp,
            tc.tile_pool(name="sml", bufs=2) as sml,
            tc.tile_pool(name="ost", bufs=3) as ost,
            tc.tile_pool(name="psS", bufs=2, space="PSUM") as psS,
            tc.tile_pool(name="psO", bufs=2, space="PSUM") as psO,
            tc.tile_pool(name="psM", bufs=2, space="PSUM") as psM,
        ):
            # ---- constants -------------------------------------------------
            wq = const.tile([P, KD, ESL], F32R, tag="wq")
            wk = const.tile([P, KD, ESL], F32R, tag="wk")
            wv = const.tile([P, KD, ESL], F32R, tag="wv")
            nc.sync.dma_start(wq[:], wqT_d[:, :].rearrange("(ko p) e -> p ko e", p=P).bitcast(F32R))
            nc.sync.dma_start(wk[:], wkT_d[:, :].rearrange("(ko p) e -> p ko e", p=P).bitcast(F32R))
            nc.sync.dma_start(wv[:], wvT_d[:, :].rearrange("(ko p) e -> p ko e", p=P).bitcast(F32R))
            wo = const.tile([ESL, D], F32R, tag="wo")
            nc.sync.dma_start(wo[:], woT_d[:, :].bitcast(F32R))
            bq = const.tile([ESL, 1], F32, tag="bq")
            bk = const.tile([ESL, 1], F32, tag="bk")
            bv = const.tile([ESL, 1], F32, tag="bv")
            nc.sync.dma_start(bq[:], bq_d[:, :])
            nc.sync.dma_start(bk[:], bk_d[:, :])
            nc.sync.dma_start(bv[:], bv_d[:, :])
            ident = const.tile([P, P], F32, tag="ident")
            make_identity(nc, ident[:])
            ones = const.tile([P, DK], F32R, tag="ones")
            nc.sync.dma_start(ones[:], ones_d[:, 0:DK].bitcast(F32R))

            # ---- persistent activations -----------------------------------
            # q/k transposed: [e_local, token]; head A dims on partitions
            # 0:64, head B on 64:128.
            qt = big.tile([P, T], F32R, tag="qt")
            kt_ = big.tile([P, T], F32R, tag="kt")
            # v in natural [token, dv] layout + ones column for the softmax
            # denominator: [128, subtile, 65]
            vnA = big.tile([P, NST, DK + 1], F32R, tag="vnA")
            vnB = big.tile([P, NST, DK + 1], F32R, tag="vnB")
            nc.sync.dma_start(vnA[:, :, DK : DK + 1], ones_d[:, :, None].bitcast(F32R))
            nc.sync.dma_start(vnB[:, :, DK : DK + 1], ones_d[:, :, None].bitcast(F32R))

            # ---- phase 1: projections -------------------------------------
            for c in range(NPCH):
                tok = c * PCH
                xt = xs.tile([P, KD, PCH], F32R, tag="xt")
                nc.sync.dma_start(
                    xt[:],
                    xT_d[:, tok : tok + PCH].rearrange("(ko p) t -> p ko t", p=P).bitcast(F32R),
                )
                pqk = psS.tile([P, 2 * PCH], F32, tag="ps")
                pv = psS.tile([P, 2 * PCH], F32, tag="ps")
                for ko in range(KD):
                    nc.tensor.matmul(pqk[:, 0:PCH], wq[:, ko, :], xt[:, ko, :],
                                     start=(ko == 0), stop=(ko == KD - 1))
                for ko in range(KD):
                    nc.tensor.matmul(pqk[:, PCH : 2 * PCH], wk[:, ko, :], xt[:, ko, :],
                                     start=(ko == 0), stop=(ko == KD - 1))
                for ko in range(KD):
                    nc.tensor.matmul(pv[:, 0:PCH], wv[:, ko, :], xt[:, ko, :],
                                     start=(ko == 0), stop=(ko == KD - 1))
                nc.vector.tensor_scalar_add(qt[:, tok : tok + PCH], pqk[:, 0:PCH], bq[:])
                nc.vector.tensor_scalar_add(kt_[:, tok : tok + PCH], pqk[:, PCH : 2 * PCH], bk[:])
                # v: bias-add into staging, then PE-transpose into natural layout
                vt = vtp.tile([P, PCH], F32, tag="vt")
                nc.vector.tensor_scalar_add(vt[:], pv[:, 0:PCH], bv[:])
                for s4 in range(PCH // P):
                    st = c * (PCH // P) + s4
                    ptA = psM.tile([P, PCH], F32, tag="psm")
                    nc.tensor.transpose(ptA[:, 0:DK], vt[0:DK, s4 * P : (s4 + 1) * P],
                                        ident[0:DK, 0:DK])
                    nc.vector.tensor_copy(vnA[:, st, 0:DK], ptA[:, 0:DK])
                    ptB = psM.tile([P, PCH], F32, tag="psm")
                    nc.tensor.transpose(ptB[:, 0:DK], vt[DK:P, s4 * P : (s4 + 1) * P],
                                        ident[DK:P, DK:P])
                    nc.vector.tensor_copy(vnB[:, st, 0:DK], ptB[:, 0:DK])

            # ---- phase 2: attention + partial out-projection per batch ----
            for b in range(B):
                ot = otp.tile([P, S], F32R, tag="ot")  # attn_out^T for batch b
                for c in range(NQC):
                    q0 = b * S + c * QC
                    nkt = (c + 1) * (QC // P)
                    poA = psO.tile([P, QC], F32, tag="po")
                    poB = psO.tile([P, QC], F32, tag="po")
                    for kt in range(nkt):
                        k0 = b * S + kt * P
                        ps = psS.tile([P, 2 * QC], F32, tag="ps")
                        nc.tensor.matmul(ps[:, 0:QC], kt_[0:DK, k0 : k0 + P],
                                         qt[0:DK, q0 : q0 + QC], start=True, stop=True)
                        nc.tensor.matmul(ps[:, QC : 2 * QC], kt_[DK:P, k0 : k0 + P],
                                         qt[DK:P, q0 : q0 + QC], start=True, stop=True)
                        at = attnp.tile([P, 2 * QC], F32R, tag="at")
                        nc.scalar.activation(at[:], ps[:], AF.Exp, bias=0.0, scale=0.125)
                        if kt >= nkt - (QC // P):  # diagonal tile: causal mask
                            at3 = at[:].rearrange("p (h q) -> p h q", h=2)
                            nc.gpsimd.affine_select(
                                out=at3, in_=at3,
                                compare_op=mybir.AluOpType.is_ge,
                                fill=0.0,
                                base=c * QC - kt * P,
                                pattern=[[0, 2], [1, QC]],
                                channel_multiplier=-1,
                            )
                        st = b * (S // P) + kt
                        nc.tensor.matmul(poA[0 : DK + 1, :], vnA[:, st, :], at[:, 0:QC],
                                         start=(kt == 0), stop=(kt == nkt - 1))
                        nc.tensor.matmul(poB[0 : DK + 1, :], vnB[:, st, :], at[:, QC : 2 * QC],
                                         start=(kt == 0), stop=(kt == nkt - 1))
                    # normalize: rows 0:64 / row 64 (the ones-column sum)
                    rcA = sml.tile([P, QC], F32R, tag="rc")
                    rcB = sml.tile([P, QC], F32R, tag="rc")
                    with nc.allow_low_precision(reason="f32r reciprocal for softmax denom"):
                        nc.vector.reciprocal(rcA[DK : DK + 1, :], poA[DK : DK + 1, :])
                        nc.vector.reciprocal(rcB[DK : DK + 1, :], poB[DK : DK + 1, :])
                    bcA = psM.tile([P, QC], F32, tag="psm")
                    bcB = psM.tile([P, QC], F32, tag="psm")
                    nc.tensor.matmul(bcA[0:DK, :], ones[DK : DK + 1, 0:DK],
                                     rcA[DK : DK + 1, :], start=True, stop=True)
                    nc.tensor.matmul(bcB[0:DK, :], ones[DK : DK + 1, 0:DK],
                                     rcB[DK : DK + 1, :], start=True, stop=True)
                    bcsA = ost.tile([P, QC], F32, tag="bcs")
                    bcsB = ost.tile([P, QC], F32, tag="bcs")
                    nc.vector.tensor_copy(bcsA[0:DK, :], bcA[0:DK, :])
                    nc.vector.tensor_copy(bcsB[0:DK, :], bcB[0:DK, :])
                    nc.vector.tensor_mul(ot[0:DK, c * QC : (c + 1) * QC],
                                         poA[0:DK, :], bcsA[0:DK, :])
                    obt = ost.tile([P, QC], F32R, tag="obt")
                    nc.vector.tensor_mul(obt[0:DK, :], poB[0:DK, :], bcsB[0:DK, :])
                    # head B rows must land on partitions 64:128 -> DMA shift
                    nc.sync.dma_start(ot[DK:P, c * QC : (c + 1) * QC], obt[0:DK, :])

                # partial out-projection for batch b
                for tt in range(S // P):
                    t0 = b * S + tt * P
                    for ec in range(D // 512):
                        pp = psM.tile([P, QC], F32, tag="psm")
                        nc.tensor.matmul(pp[:], ot[:, tt * P : (tt + 1) * P],
                                         wo[:, ec * 512 : (ec + 1) * 512],
                                         start=True, stop=True)
                        ob = ost.tile([P, 512], F32, tag="ob")
                        nc.vector.tensor_copy(ob[:], pp[:])
                        nc.sync.dma_start(out_d[t0 : t0 + P, ec * 512 : (ec + 1) * 512], ob[:])

    nc.compile()
    return nc


_ONES = np.ones((P, NST), dtype=np.float32)
_NC = None


def _get_nc():
    global _NC
    if _NC is None:
        _NC = _build()
    return _NC


def _install_profile_shim():
    """Provide antenv.axon_hooks + ctypes NTFF hook (missing in this image)."""
    if "antenv.axon_hooks" in sys.modules:
        return
    so_path = "/opt/axon/libaxon_pjrt.so"
    state = {"hook": None}

    def set_hook(h):
        state["hook"] = h

    def get_hook():
        return state["hook"]

    mod = types.ModuleType("antenv.axon_hooks")
    mod.set_axon_ntff_profile_hook = set_hook
    mod.get_axon_ntff_profile_hook = get_hook
    sys.modules["antenv.axon_hooks"] = mod
    try:
        lib = ctypes.CDLL(so_path)
        if not hasattr(lib, "axon_start_nrt_profile"):
            return
        lib.axon_start_nrt_profile.argtypes = [ctypes.POINTER(ctypes.c_int64), ctypes.c_size_t]
        lib.axon_start_nrt_profile.restype = ctypes.c_int64
        lib.axon_stop_nrt_profile.argtypes = [ctypes.c_char_p]
        lib.axon_stop_nrt_profile.restype = ctypes.c_int64

        @contextlib.contextmanager
        def _hook_cm(output_dir, device_ids):
            import jax

            jax.devices()
            if device_ids:
                ids = (ctypes.c_int64 * len(device_ids))(*device_ids)
                rc = lib.axon_start_nrt_profile(ids, len(device_ids))
            else:
                rc = lib.axon_start_nrt_profile(None, 0)
            if rc != 0:
                raise RuntimeError(f"axon_start_nrt_profile rc={rc}")
            try:
                yield
            finally:
                n = lib.axon_stop_nrt_profile(str(output_dir).encode())
                print(f"ntff profile: {n} file(s) in {output_dir}", file=sys.stderr)

        set_hook(_hook_cm)
    except OSError:
        pass


def run(inputs, trace=False):
    """Run the kernel; returns (full_output, BassKernelResults)."""
    x = np.ascontiguousarray(inputs["x"], dtype=np.float32)
    Wq = np.asarray(inputs["Wq"], dtype=np.float32)
    Wk = np.asarray(inputs["Wk"], dtype=np.float32)
    Wv = np.asarray(inputs["Wv"], dtype=np.float32)
    Wo = np.asarray(inputs["Wo"], dtype=np.float32)
    bq = np.asarray(inputs["bq"], dtype=np.float32)
    bk = np.asarray(inputs["bk"], dtype=np.float32)
    bv = np.asarray(inputs["bv"], dtype=np.float32)
    bo = np.asarray(inputs["bo"], dtype=np.float32)

    xT = np.ascontiguousarray(x.reshape(T, D).T)
    in_maps = []
    for j in range(NCORES):
        sl = slice(ESL * j, ESL * (j + 1))
        in_maps.append({
            "xT": xT,
            "wqT": np.ascontiguousarray(Wq[sl, :].T),
            "wkT": np.ascontiguousarray(Wk[sl, :].T),
            "wvT": np.ascontiguousarray(Wv[sl, :].T),
            "bq": np.ascontiguousarray(bq[sl].reshape(ESL, 1)),
            "bk": np.ascontiguousarray(bk[sl].reshape(ESL, 1)),
            "bv": np.ascontiguousarray(bv[sl].reshape(ESL, 1)),
            "woT": np.ascontiguousarray(Wo[:, sl].T),
            "onesb": _ONES,
        })

    if trace:
        _install_profile_shim()
    nc = _get_nc()
    res = run_bass_kernel_spmd(nc, in_maps, core_ids=list(range(NCORES)), trace=trace)
    acc = res.results[0]["outp"].astype(np.float32).copy()
    for j in range(1, NCORES):
        acc += res.results[j]["outp"]
    acc += bo[None, :]
    return acc.reshape(B, S, D), res


def kernel(**inputs):
    out, _ = run(inputs, trace=False)
    return out


if __name__ == "__main__":
    import reference

    ins = {k: np.asarray(v) for k, v in reference.setup_inputs().items()}
    out, res = run(ins, trace=False)
    exp = np.asarray(reference.reference(**ins))
    err = np.abs(out - exp)
    denom = np.abs(exp).max()
    print("absmax ref:", denom, "absmax err:", err.max(),
          "rel:", err.max() / denom)


# revision 12
# speedup vs baseline: 1.3656x; 1.3656x over previous
"""Causal multi-head self-attention on 8 trn2 NeuronCores.

Problem (hardcoded): x [4, 2048, 1024] fp32, 16 heads, d_k = 64.
    q/k/v = x @ W{q,k,v}.T + b     (per-head split)
    attn  = softmax(causal(q k^T / 8)) @ v
    out   = concat_heads(attn) @ Wo.T + bo

Sharding: tensor-parallel over heads. Core j owns heads {2j, 2j+1} =
feature slice [128j, 128j+128). Each core:
  1. Q/K/V projections for its slice, all 8192 tokens (output kept
     transposed: [e_local, token] in SBUF).
  2. Flash-style causal attention for its 8 (batch, head) pairs,
     computed in scores^T layout ([key, query]) so the attention
     weights land contraction-major for the V matmul. Softmax
     denominator comes from a ones-column appended to V.
  3. Partial output projection  attn_out_slice @ Wo[:, slice].T
     -> [8192, 1024] partial, summed on the host over the 8 cores.

All matmuls run as float32r (TF32-like, full PE rate at N>=512).
Work is interleaved per batch so projections of batch b+1 overlap the
(ACT-bound) attention of batch b; diagonal score tiles are processed
first within each query chunk so the causal-mask latency hides behind
the unmasked tiles.
"""

import contextlib
import ctypes
import sys
import types

import numpy as np

import concourse.bacc as bacc
import concourse.bass as bass
import concourse.mybir as mybir
import concourse.tile as tile
from concourse.bass_utils import run_bass_kernel_spmd
from concourse.masks import make_identity

F32 = mybir.dt.float32
F32R = mybir.dt.float32r
AF = mybir.ActivationFunctionType

B, S, D = 4, 2048, 1024
H, DK = 16, 64
T = B * S              # 8192 tokens
P = 128                # partitions
NCORES = 8
ESL = D // NCORES      # 128 = per-core feature slice (2 heads)
KD = D // P            # 8 k-tiles for the projections
PCH = 512              # projection token-chunk
NPCB = S // PCH        # 4 chunks per batch
QC = 512               # query chunk in attention
NQC = S // QC          # 4 per batch
NSB = S // P           # 16 token subtiles per batch (V natural layout)


def _build():
    nc = bacc.Bacc("TRN2", target_bir_lowering=False, debug=False)

    xT_d = nc.dram_tensor("xT", (D, T), F32, kind="ExternalInput")
    wqT_d = nc.dram_tensor("wqT", (D, ESL), F32, kind="ExternalInput")
    wkT_d = nc.dram_tensor("wkT", (D, ESL), F32, kind="ExternalInput")
    wvT_d = nc.dram_tensor("wvT", (D, ESL), F32, kind="ExternalInput")
    bq_d = nc.dram_tensor("bq", (ESL, 1), F32, kind="ExternalInput")
    bk_d = nc.dram_tensor("bk", (ESL, 1), F32, kind="ExternalInput")
    bv_d = nc.dram_tensor("bv", (ESL, 1), F32, kind="ExternalInput")
    woT_d = nc.dram_tensor("woT", (ESL, D), F32, kind="ExternalInput")
    ones_d = nc.dram_tensor("onesb", (P, NSB), F32, kind="ExternalInput")
    out_d = nc.dram_tensor("outp", (T, D), F32, kind="ExternalOutput")

    with tile.TileContext(nc) as tc:
        with (
            tc.tile_pool(name="const", bufs=1) as const,
            tc.tile_pool(name="xs", bufs=2) as xs,
            tc.tile_pool(name="qkp", bufs=1) as qkp,
            tc.tile_pool(name="vnp", bufs=1) as vnp,
            tc.tile_pool(name="vt", bufs=2) as vtp,
            tc.tile_pool(name="attn", bufs=4) as attnp,
            tc.tile_pool(name="otp", bufs=2) as otp,
            tc.tile_pool(name="sml", bufs=2) as sml,
            tc.tile_pool(name="ost", bufs=3) as ost,
            tc.tile_pool(name="psS", bufs=2, space="PSUM") as psS,
            tc.tile_pool(name="psO", bufs=2, space="PSUM") as psO,
            tc.tile_pool(name="psM", bufs=2, space="PSUM") as psM,
        ):
            # ---- constants -------------------------------------------------
            wq = const.tile([P, KD, ESL], F32R, tag="wq")
            wk = const.tile([P, KD, ESL], F32R, tag="wk")
            wv = const.tile([P, KD, ESL], F32R, tag="wv")
            nc.sync.dma_start(wq[:], wqT_d[:, :].rearrange("(ko p) e -> p ko e", p=P).bitcast(F32R))
            nc.sync.dma_start(wk[:], wkT_d[:, :].rearrange("(ko p) e -> p ko e", p=P).bitcast(F32R))
            nc.sync.dma_start(wv[:], wvT_d[:, :].rearrange("(ko p) e -> p ko e", p=P).bitcast(F32R))
            wo = const.tile([ESL, D], F32R, tag="wo")
            nc.sync.dma_start(wo[:], woT_d[:, :].bitcast(F32R))
            bq = const.tile([ESL, 1], F32, tag="bq")
            bk = const.tile([ESL, 1], F32, tag="bk")
            bv = const.tile([ESL, 1], F32, tag="bv")
            nc.sync.dma_start(bq[:], bq_d[:, :])
            nc.sync.dma_start(bk[:], bk_d[:, :])
            nc.sync.dma_start(bv[:], bv_d[:, :])
            ident = const.tile([P, P], F32, tag="ident")
            make_identity(nc, ident[:])

            # per-batch persistent activations (lets batch b+1 projections
            # overlap batch b attention)
            qts, kts, vnAs, vnBs = [], [], [], []
            for b in range(B):
                qts.append(qkp.tile([P, S], F32R, tag=f"qt{b % 2}", name=f"qt_b{b}"))
                kts.append(qkp.tile([P, S], F32R, tag=f"kt{b % 2}", name=f"kt_b{b}"))
                vnAs.append(vnp.tile([P, NSB, DK + 1], F32R, tag=f"vnA{b % 2}", name=f"vnA_b{b}"))
                vnBs.append(vnp.tile([P, NSB, DK + 1], F32R, tag=f"vnB{b % 2}", name=f"vnB_b{b}"))

            def proj_batch(b):
                """Q/K/V projections for batch b's 2048 tokens."""
                qt, kt_, vnA, vnB = qts[b], kts[b], vnAs[b], vnBs[b]
                nc.sync.dma_start(vnA[:, :, DK : DK + 1], ones_d[:, 0:NSB, None].bitcast(F32R))
                nc.sync.dma_start(vnB[:, :, DK : DK + 1], ones_d[:, 0:NSB, None].bitcast(F32R))
                for c in range(NPCB):
                    tok = b * S + c * PCH
                    loc = c * PCH
                    xt = xs.tile([P, KD, PCH], F32R, tag="xt")
                    nc.sync.dma_start(
                        xt[:],
                        xT_d[:, tok : tok + PCH].rearrange("(ko p) t -> p ko t", p=P).bitcast(F32R),
                    )
                    pqk = psS.tile([P, 2 * PCH], F32, tag="ps")
                    pv = psS.tile([P, 2 * PCH], F32, tag="ps")
                    for ko in range(KD):
                        nc.tensor.matmul(pqk[:, 0:PCH], wq[:, ko, :], xt[:, ko, :],
                                         start=(ko == 0), stop=(ko == KD - 1))
                    for ko in range(KD):
                        nc.tensor.matmul(pqk[:, PCH : 2 * PCH], wk[:, ko, :], xt[:, ko, :],
                                         start=(ko == 0), stop=(ko == KD - 1))
                    for ko in range(KD):
                        nc.tensor.matmul(pv[:, 0:PCH], wv[:, ko, :], xt[:, ko, :],
                                         start=(ko == 0), stop=(ko == KD - 1))
                    nc.vector.tensor_scalar_add(qt[:, loc : loc + PCH], pqk[:, 0:PCH], bq[:])
                    nc.vector.tensor_scalar_add(kt_[:, loc : loc + PCH], pqk[:, PCH : 2 * PCH], bk[:])
                    # v: bias-add to staging, then PE-transpose to [token, dv]
                    vt = vtp.tile([P, PCH], F32, tag="vt")
                    nc.vector.tensor_scalar_add(vt[:], pv[:, 0:PCH], bv[:])
                    for s4 in range(PCH // P):
                        st = c * (PCH // P) + s4
                        ptA = psM.tile([P, PCH], F32, tag="psm")
                        nc.tensor.transpose(ptA[:, 0:DK], vt[0:DK, s4 * P : (s4 + 1) * P],
                                            ident[0:DK, 0:DK])
                        nc.vector.tensor_copy(vnA[:, st, 0:DK], ptA[:, 0:DK])
                        ptB = psM.tile([P, PCH], F32, tag="psm")
                        nc.tensor.transpose(ptB[:, 0:DK], vt[DK:P, s4 * P : (s4 + 1) * P],
                                            ident[DK:P, DK:P])
                        nc.vector.tensor_copy(vnB[:, st, 0:DK], ptB[:, 0:DK])

            def attn_batch(b):
                """Attention + partial out-projection for batch b."""
                qt, kt_, vnA, vnB = qts[b], kts[b], vnAs[b], vnBs[b]
                ot = otp.tile([P, S], F32R, tag="ot")  # attn_out^T
                for c in range(NQC):
                    q0 = c * QC
                    nkt = (c + 1) * (QC // P)
                    ndiag = QC // P
                    # diagonal (masked) k-tiles first so the mask latency
                    # hides behind the unmasked tiles' S/exp work
                    kt_order = list(range(nkt - ndiag, nkt)) + list(range(nkt - ndiag))
                    poA = psO.tile([P, QC], F32, tag="po")
                    poB = psO.tile([P, QC], F32, tag="po")
                    for i, kt in enumerate(kt_order):
                        k0 = kt * P
                        ps = psS.tile([P, 2 * QC], F32, tag="ps")
                        nc.tensor.matmul(ps[:, 0:QC], kt_[0:DK, k0 : k0 + P],
                                         qt[0:DK, q0 : q0 + QC], start=True, stop=True)
                        nc.tensor.matmul(ps[:, QC : 2 * QC], kt_[DK:P, k0 : k0 + P],
                                         qt[DK:P, q0 : q0 + QC], start=True, stop=True)
                        at = attnp.tile([P, 2 * QC], F32R, tag="at")
                        nc.scalar.activation(at[:], ps[:], AF.Exp, bias=0.0, scale=0.125)
                        if kt >= nkt - ndiag:  # diagonal tile: causal mask
                            at3 = at[:].rearrange("p (h q) -> p h q", h=2)
                            nc.gpsimd.affine_select(
                                out=at3, in_=at3,
                                compare_op=mybir.AluOpType.is_ge,
                                fill=0.0,
                                base=c * QC - kt * P,
                                pattern=[[0, 2], [1, QC]],
                                channel_multiplier=-1,
                            )
                        st = kt
                        nc.tensor.matmul(poA[0 : DK + 1, :], vnA[:, st, :], at[:, 0:QC],
                                         start=(i == 0), stop=(i == nkt - 1))
                        nc.tensor.matmul(poB[0 : DK + 1, :], vnB[:, st, :], at[:, QC : 2 * QC],
                                         start=(i == 0), stop=(i == nkt - 1))
                    # normalize: rows 0:64 * recip(row 64). The denom rows
                    # sit on partition 64; shift to partition 0 via DMA so
                    # partition_broadcast (which only reads partition 0) works.
                    den = sml.tile([P, 2 * QC], F32, tag="den")
                    nc.vector.tensor_copy(den[DK : DK + 1, 0:QC], poA[DK : DK + 1, :])
                    nc.vector.tensor_copy(den[DK : DK + 1, QC : 2 * QC], poB[DK : DK + 1, :])
                    nc.sync.dma_start(den[0:1, :], den[DK : DK + 1, :])
                    rc = sml.tile([P, 2 * QC], F32, tag="rc")
                    nc.vector.reciprocal_approx_fast(rc[0:1, :], den[0:1, :])
                    bc = sml.tile([DK, 2 * QC], F32, tag="bc")
                    nc.gpsimd.partition_broadcast(bc[:], rc[0:1, :], channels=DK)
                    nc.vector.tensor_mul(ot[0:DK, q0 : q0 + QC], poA[0:DK, :], bc[:, 0:QC])
                    obt = ost.tile([P, QC], F32R, tag="obt")
                    nc.vector.tensor_mul(obt[0:DK, :], poB[0:DK, :], bc[:, QC : 2 * QC])
                    # head B rows land on partitions 64:128 via DMA shift
                    nc.sync.dma_start(ot[DK:P, q0 : q0 + QC], obt[0:DK, :])

                # partial out-projection for batch b
                for tt in range(S // P):
                    t0 = b * S + tt * P
                    for ec in range(D // 512):
                        pp = psM.tile([P, 512], F32, tag="psm")
                        nc.tensor.matmul(pp[:], ot[:, tt * P : (tt + 1) * P],
                                         wo[:, ec * 512 : (ec + 1) * 512],
                                         start=True, stop=True)
                        ob = ost.tile([P, 512], F32, tag="ob")
                        nc.vector.tensor_copy(ob[:], pp[:])
                        nc.sync.dma_start(out_d[t0 : t0 + P, ec * 512 : (ec + 1) * 512], ob[:])

            for b in range(B):
                with nc.named_scope(f"proj_b{b}"):
                    proj_batch(b)
                if b > 0:
                    with nc.named_scope(f"attn_b{b - 1}"):
                        attn_batch(b - 1)
            with nc.named_scope(f"attn_b{B - 1}"):
                attn_batch(B - 1)

    nc.compile()
    return nc


_ONES = np.ones((P, NSB), dtype=np.float32)
_NC = None


def _get_nc():
    global _NC
    if _NC is None:
        _NC = _build()
    return _NC


def _install_profile_shim():
    """Provide antenv.axon_hooks + ctypes NTFF hook (missing in this image)."""
    if "antenv.axon_hooks" in sys.modules:
        return
    so_path = "/opt/axon/libaxon_pjrt.so"
    state = {"hook": None}

    def set_hook(h):
        state["hook"] = h

    def get_hook():
        return state["hook"]

    mod = types.ModuleType("antenv.axon_hooks")
    mod.set_axon_ntff_profile_hook = set_hook
    mod.get_axon_ntff_profile_hook = get_hook
    sys.modules["antenv.axon_hooks"] = mod
    try:
        lib = ctypes.CDLL(so_path)
        if not hasattr(lib, "axon_start_nrt_profile"):
            return
        lib.axon_start_nrt_profile.argtypes = [ctypes.POINTER(ctypes.c_int64), ctypes.c_size_t]
        lib.axon_start_nrt_profile.restype = ctypes.c_int64
        lib.axon_stop_nrt_profile.argtypes = [ctypes.c_char_p]
        lib.axon_stop_nrt_profile.restype = ctypes.c_int64

        @contextlib.contextmanager
        def _hook_cm(output_dir, device_ids):
            import jax

            jax.devices()
            if device_ids:
                ids = (ctypes.c_int64 * len(device_ids))(*device_ids)
                rc = lib.axon_start_nrt_profile(ids, len(device_ids))
            else:
                rc = lib.axon_start_nrt_profile(None, 0)
            if rc != 0:
                raise RuntimeError(f"axon_start_nrt_profile rc={rc}")
            try:
                yield
            finally:
                n = lib.axon_stop_nrt_profile(str(output_dir).encode())
                print(f"ntff profile: {n} file(s) in {output_dir}", file=sys.stderr)

        set_hook(_hook_cm)
    except OSError:
        pass


def run(inputs, trace=False):
    """Run the kernel; returns (full_output, BassKernelResults)."""
    x = np.ascontiguousarray(inputs["x"], dtype=np.float32)
    Wq = np.asarray(inputs["Wq"], dtype=np.float32)
    Wk = np.asarray(inputs["Wk"], dtype=np.float32)
    Wv = np.asarray(inputs["Wv"], dtype=np.float32)
    Wo = np.asarray(inputs["Wo"], dtype=np.float32)
    bq = np.asarray(inputs["bq"], dtype=np.float32)
    bk = np.asarray(inputs["bk"], dtype=np.float32)
    bv = np.asarray(inputs["bv"], dtype=np.float32)
    bo = np.asarray(inputs["bo"], dtype=np.float32)

    xT = np.ascontiguousarray(x.reshape(T, D).T)
    in_maps = []
    for j in range(NCORES):
        sl = slice(ESL * j, ESL * (j + 1))
        in_maps.append({
            "xT": xT,
            "wqT": np.ascontiguousarray(Wq[sl, :].T),
            "wkT": np.ascontiguousarray(Wk[sl, :].T),
            "wvT": np.ascontiguousarray(Wv[sl, :].T),
            "bq": np.ascontiguousarray(bq[sl].reshape(ESL, 1)),
            "bk": np.ascontiguousarray(bk[sl].reshape(ESL, 1)),
            "bv": np.ascontiguousarray(bv[sl].reshape(ESL, 1)),
            "woT": np.ascontiguousarray(Wo[:, sl].T),
            "onesb": _ONES,
        })

    if trace:
        _install_profile_shim()
    nc = _get_nc()
    res = run_bass_kernel_spmd(nc, in_maps, core_ids=list(range(NCORES)), trace=trace)
    acc = res.results[0]["outp"].astype(np.float32).copy()
    for j in range(1, NCORES):
        acc += res.results[j]["outp"]
    acc += bo[None, :]
    return acc.reshape(B, S, D), res


def kernel(**inputs):
    out, _ = run(inputs, trace=False)
    return out


if __name__ == "__main__":
    import reference

    ins = {k: np.asarray(v) for k, v in reference.setup_inputs().items()}
    out, res = run(ins, trace=False)
    exp = np.asarray(reference.reference(**ins))
    err = np.abs(out - exp)
    denom = np.abs(exp).max()
    print("absmax ref:", denom, "absmax err:", err.max(),
          "rel:", err.max() / denom)
